# revision 8
# baseline (speedup 1.0000x reference)
"""Trainium2 Bass kernel for nn_AttentionClassificationHead.

Windowed single-query multi-head attention (T=6 tokens/window, H=4 heads,
E=256) + classifier head (LayerNorm -> Linear -> GELU -> Linear), data
parallel over 8 NeuronCores (batch axis sharded, 8 batches/core).

Host-folded exact rewrites:
  - scores_{t,h} = x_t . qk_h,  qk = einsum('hd,hde->eh', q, Wk)/sqrt(hd);
    per-head score bias (q.bk) cancels in softmax and is dropped.
  - attn = Wv @ xbar + bv,  xbar_h = sum_t attw_{h,t} x_t  (V-proj after the
    attention average); out = out_w @ attn + (out_w@bv + out_b).
  - LayerNorm gain/bias folded into first MLP layer: W1' = w1*ln_g,
    b1' = b1 + w1@ln_b.
"""
import numpy as np
import ml_dtypes
from contextlib import ExitStack

T = 6
H = 4
B = 64
N = 512
E = 256
HD = E // H
NCORES = 8
BPC = B // NCORES            # batches per core
LW = N * BPC                 # windows per core  = 4096
LT = LW * T                  # tokens per core   = 24576
TT = 126                     # tokens per full tile
TW = 21                      # windows per full tile
NFULL = LT // TT             # 195 full tiles
REM = LT - NFULL * TT        # 6 -> one mini tile (1 window)
SG_TILES = 24                # tiles per supergroup (504 windows)
NMCOL = NFULL + (1 if REM else 0)

BF = ml_dtypes.bfloat16


def _tile_sizes():
    ts = [(TT, TW)] * NFULL
    if REM:
        ts.append((REM, REM // T))
    return ts


def _build_consts(query, in_proj_w, in_proj_b, out_w, out_b, ln_g, ln_b,
                  w1, b1, w2, b2):
    Wq, Wk, Wv = in_proj_w[:E], in_proj_w[E:2 * E], in_proj_w[2 * E:]
    bq, bk, bv = in_proj_b[:E], in_proj_b[E:2 * E], in_proj_b[2 * E:]
    q = (query @ Wq.T + bq).reshape(H, HD)
    qk = (np.einsum('hd,hde->eh', q, Wk.reshape(H, HD, E))
          / np.float32(np.sqrt(HD))).astype(np.float32)      # (E, H)
    cvec = (out_w @ bv + out_b).astype(np.float32)           # (E,)
    W1p = (w1 * ln_g[None, :]).astype(np.float32)            # (128, 256)
    b1p = (b1 + w1 @ ln_b).astype(np.float32)                # (128,)

    ob = np.zeros((TT, TW), np.float32)
    for w in range(TW):
        ob[w * T:(w + 1) * T, w] = 1.0

    c = {
        "ident": np.eye(128, dtype=BF),
        "qk": np.concatenate([qk[:128], qk[128:]], axis=1).astype(BF),
        "onesblk": ob,
        "onesblkT": np.ascontiguousarray(ob.T),
        "onesmask6": np.ascontiguousarray(
            np.tile(np.repeat(ob, H, axis=1), (1, 6))),
        "WvT": np.concatenate([Wv.T[:128], Wv.T[128:]], axis=1).astype(BF),
        "OwT": np.concatenate([out_w.T[:128], out_w.T[128:]],
                              axis=1).astype(BF),
        "W1T": np.concatenate([W1p.T[:128], W1p.T[128:]], axis=1).astype(BF),
        "w2": np.ascontiguousarray(w2.reshape(E // 2, 1)).astype(BF),
        "cvec": np.ascontiguousarray(np.stack([cvec[:128], cvec[128:]],
                                              axis=1)),
        "b1v": np.ascontiguousarray(b1p.reshape(E // 2, 1)),
        "oneE": np.full((128, 1), 1.0 / E, np.float32),
        "one1": np.ones((1, 128), np.float32),
    }
    return c, float(np.asarray(b2).reshape(-1)[0])


CONST_SPECS = [
    ("ident", [128, 128], "bf16"), ("qk", [128, 8], "bf16"),
    ("onesblk", [TT, TW], "f32"), ("onesblkT", [TW, TT], "f32"),
    ("onesmask6", [TT, TW * H * 6], "f32"),
    ("WvT", [128, 512], "bf16"), ("OwT", [128, 512], "bf16"),
    ("W1T", [128, 256], "bf16"), ("w2", [128, 1], "bf16"),
    ("cvec", [128, 2], "f32"), ("b1v", [128, 1], "f32"),
    ("oneE", [128, 1], "f32"), ("one1", [1, 128], "f32"),
]


def _emit(tc, d, out, b2f):
    from concourse import mybir
    nc = tc.nc
    f32 = mybir.dt.float32
    bf16 = mybir.dt.bfloat16
    AF = mybir.ActivationFunctionType
    OP = mybir.AluOpType

    sizes = _tile_sizes()
    ntiles = len(sizes)
    sgs = []
    i = 0
    while i < ntiles:
        sgs.append(list(range(i, min(i + SG_TILES, ntiles))))
        i += SG_TILES

    ctx = ExitStack()
    with ctx:
        cpool = ctx.enter_context(tc.tile_pool(name="consts", bufs=1))
        xwp = ctx.enter_context(tc.tile_pool(name="xw", bufs=2))
        xtp = ctx.enter_context(tc.tile_pool(name="xt", bufs=4))
        smp = ctx.enter_context(tc.tile_pool(name="sm", bufs=2))
        blkp = ctx.enter_context(tc.tile_pool(name="blk", bufs=2))
        xbgp = ctx.enter_context(tc.tile_pool(name="xbg", bufs=2))
        tailp = ctx.enter_context(tc.tile_pool(name="tail", bufs=2))
        psp = ctx.enter_context(tc.tile_pool(name="ps", bufs=8, space="PSUM"))

        cs = {}
        for name, shape, dt in CONST_SPECS:
            t = cpool.tile(shape, bf16 if dt == "bf16" else f32, tag=name)
            nc.sync.dma_start(t[:], d[name][:])
            cs[name] = t
        mco = cpool.tile([128, NMCOL * 4], f32, tag="mcols")
        nc.sync.dma_start(mco[:], d["mcols"][:])
        vld = cpool.tile([1, LW], f32, tag="valid")
        nc.sync.dma_start(vld[:], d["valid"][:])

        for sg in sgs:
            ntok_list = [sizes[j][0] for j in sg]
            nw_list = [sizes[j][1] for j in sg]
            tok0 = sg[0] * TT
            nt_sg = len(sg)
            nw_sg = sum(nw_list)
            w0 = sg[0] * TW
            full = [j for j in sg if sizes[j][0] == TT]
            nfull = len(full)

            # ---- stage 0: cast-DMA x chunk -> bf16 token-major wide tile ---
            xw = xwp.tile([TT, nt_sg * E], bf16, tag="xw")
            xwv = xw[:].rearrange("p (j e) -> p j e", e=E)
            src = d["x"][tok0:tok0 + nfull * TT, :].rearrange(
                "(j t) e -> t j e", t=TT)
            nc.gpsimd.dma_start(xwv[:, 0:nfull, :], src)
            if nt_sg > nfull:
                nc.gpsimd.dma_start(
                    xw[0:REM, nfull * E:(nfull + 1) * E],
                    d["x"][tok0 + nfull * TT:tok0 + nfull * TT + REM, :])

            def xj(jj):
                return xw[:, jj * E:(jj + 1) * E]

            xbg = [xbgp.tile([128, nw_sg * H], bf16, tag=f"xbg{c}",
                             name=f"xbg{c}") for c in range(2)]

            # ---- per softmax group (up to 6 tiles) ----
            wcol = 0
            for gs in range(0, nt_sg, 6):
                gn = min(6, nt_sg - gs)
                gtoks = ntok_list[gs:gs + gn]
                gnws = nw_list[gs:gs + gn]

                # transpose + evac + scores for the group's tiles
                s_ps = psp.tile([128, 4 * gn], f32, tag="ps")
                for gj in range(gn):
                    jj = gs + gj
                    ntok = gtoks[gj]
                    for ch in range(2):
                        xt_ps = psp.tile([128, TT], bf16, tag="ps")
                        nc.tensor.matmul(
                            xt_ps[:, 0:ntok],
                            xj(jj)[0:ntok, ch * 128:(ch + 1) * 128],
                            cs["ident"][0:ntok, 0:ntok],
                            is_transpose=True)
                        xt_sb = xtp.tile([128, 128], bf16, tag="xt")
                        if ch == 0:
                            nc.vector.tensor_copy(xt_sb[:, 0:ntok],
                                                  xt_ps[:, 0:ntok])
                        else:
                            nc.scalar.copy(xt_sb[:, 0:ntok],
                                           xt_ps[:, 0:ntok])
                        nc.tensor.matmul(
                            s_ps[:, 4 * gj:4 * gj + 4],
                            xt_sb[:],
                            cs["qk"][:, ch * 4:(ch + 1) * 4],
                            start=(ch == 0), stop=(ch == 1))

                # softmax (token-major)
                e_m = smp.tile([TT, 4 * 6], f32, tag="em")
                nc.scalar.activation(e_m[:, 0:4 * gn], s_ps[0:TT, :], AF.Exp)
                mc = mco[0:TT, (sg[0] + gs) * 4: (sg[0] + gs + gn) * 4]
                nc.vector.tensor_tensor(e_m[:, 0:4 * gn], e_m[:, 0:4 * gn],
                                        mc, OP.mult)
                d_ps = psp.tile([TW, 4 * 6], f32, tag="ps")
                nc.tensor.matmul(d_ps[:, 0:4 * gn], cs["onesblk"][:],
                                 e_m[:, 0:4 * gn], start=True, stop=True)
                dr = smp.tile([TW, 4 * 6], f32, tag="dr")
                nc.vector.tensor_scalar_add(dr[:, 0:4 * gn],
                                            d_ps[:, 0:4 * gn], 1e-30)
                nc.vector.reciprocal(dr[:, 0:4 * gn], dr[:, 0:4 * gn])
                r_ps = psp.tile([TT, 4 * 6], f32, tag="ps")
                nc.tensor.matmul(r_ps[:, 0:4 * gn], cs["onesblkT"][:],
                                 dr[:, 0:4 * gn], start=True, stop=True)
                awt = smp.tile([TT, 4 * 6], f32, tag="aw")
                nc.vector.tensor_tensor(awt[:, 0:4 * gn], e_m[:, 0:4 * gn],
                                        r_ps[:, 0:4 * gn], OP.mult)

                # block-diagonal attention weights for the group
                rep = smp.tile([TT, TW * H * 6], f32, tag="rep")
                nc.vector.tensor_scalar_mul(
                    rep[:].rearrange("p (j w h) -> p j w h",
                                     w=TW, h=4)[:, 0:gn],
                    awt[:].rearrange("p (j h) -> p j h", h=4)[:, 0:gn]
                        .unsqueeze(2).broadcast_to([TT, gn, TW, 4]),
                    1.0)
                bt = blkp.tile([TT, TW * H * 6], bf16, tag="blk")
                nc.vector.tensor_tensor(bt[:, 0:gn * TW * H],
                                        rep[:, 0:gn * TW * H],
                                        cs["onesmask6"][:, 0:gn * TW * H],
                                        OP.mult)

                # xbar for the group's tiles -> supergroup buffer
                for gj in range(gn):
                    jj = gs + gj
                    ntok = gtoks[gj]
                    nw = gnws[gj]
                    for ch in range(2):
                        xb_ps = psp.tile([128, TW * H], f32, tag="ps")
                        nc.tensor.matmul(
                            xb_ps[:, 0:nw * H],
                            xj(jj)[0:ntok, ch * 128:(ch + 1) * 128],
                            bt[0:ntok, gj * TW * H: gj * TW * H + nw * H],
                            start=True, stop=True)
                        dst = xbg[ch][:, wcol * H: (wcol + nw) * H]
                        if ch == 0:
                            nc.vector.tensor_copy(dst, xb_ps[:, 0:nw * H])
                        else:
                            nc.scalar.copy(dst, xb_ps[:, 0:nw * H])
                    wcol += nw

            # ---- tail: V-proj per head ----
            attn_sb = []
            for pair in range(2):
                a_ps = psp.tile([128, nw_sg], f32, tag="ps")
                for h2 in range(2):
                    h = pair * 2 + h2
                    for ch in range(2):
                        nc.tensor.matmul(
                            a_ps[h2 * 64:(h2 + 1) * 64, 0:nw_sg],
                            cs["WvT"][:, ch * 256 + h * 64:
                                      ch * 256 + (h + 1) * 64],
                            xbg[ch][:].rearrange("p (w h) -> p w h",
                                                 h=4)[:, :, h],
                            start=(ch == 0), stop=(ch == 1))
                a_sb = tailp.tile([128, nw_sg], bf16, tag=f"attn{pair}")
                nc.scalar.activation(a_sb[:], a_ps[:], AF.Identity,
                                     bias=cs["cvec"][:, pair:pair + 1])
                attn_sb.append(a_sb)

            # ---- out-proj ----
            out2_sb, sq_sb = [], []
            for oc in range(2):
                o_ps = psp.tile([128, nw_sg], f32, tag="ps")
                for ch in range(2):
                    nc.tensor.matmul(
                        o_ps[:], cs["OwT"][:, ch * 256 + oc * 128:
                                           ch * 256 + (oc + 1) * 128],
                        attn_sb[ch][:], start=(ch == 0), stop=(ch == 1))
                o_sb = tailp.tile([128, nw_sg], f32, tag=f"out2{oc}")
                nc.scalar.copy(o_sb[:], o_ps[:])
                s_sb = tailp.tile([128, nw_sg], f32, tag=f"sq{oc}")
                nc.scalar.square(s_sb[:], o_ps[:])
                out2_sb.append(o_sb)
                sq_sb.append(s_sb)

            # ---- LayerNorm stats ----
            mu_ps = psp.tile([1, nw_sg], f32, tag="ps")
            for ch in range(2):
                nc.tensor.matmul(mu_ps[:], cs["oneE"][:], out2_sb[ch][:],
                                 start=(ch == 0), stop=(ch == 1))
            vs_ps = psp.tile([1, nw_sg], f32, tag="ps")
            for ch in range(2):
                nc.tensor.matmul(vs_ps[:], cs["oneE"][:], sq_sb[ch][:],
                                 start=(ch == 0), stop=(ch == 1))
            mu_sb = tailp.tile([1, nw_sg], f32, tag="mu")
            nc.vector.tensor_copy(mu_sb[:], mu_ps[:])
            musq_sb = tailp.tile([1, nw_sg], f32, tag="musq")
            nc.scalar.square(musq_sb[:], mu_sb[:])
            var_sb = tailp.tile([1, nw_sg], f32, tag="var")
            nc.vector.scalar_tensor_tensor(var_sb[:], vs_ps[:], 1e-5,
                                           musq_sb[:], OP.add, OP.subtract)
            std_sb = tailp.tile([1, nw_sg], f32, tag="std")
            nc.scalar.activation(std_sb[:], var_sb[:], AF.Sqrt, bias=0.0)
            rstd_sb = tailp.tile([1, nw_sg], f32, tag="rstd")
            nc.vector.reciprocal(rstd_sb[:], std_sb[:])
            nmu_sb = tailp.tile([1, nw_sg], f32, tag="nmu")
            nc.vector.scalar_tensor_tensor(nmu_sb[:], mu_sb[:], -1.0,
                                           rstd_sb[:], OP.mult, OP.mult)
            rstd_ps = psp.tile([128, nw_sg], f32, tag="ps")
            nc.tensor.matmul(rstd_ps[:], cs["one1"][:], rstd_sb[:],
                             start=True, stop=True)
            nmu_ps = psp.tile([128, nw_sg], f32, tag="ps")
            nc.tensor.matmul(nmu_ps[:], cs["one1"][:], nmu_sb[:],
                             start=True, stop=True)

            # ---- normalize + MLP ----
            h1_ps = psp.tile([128, nw_sg], f32, tag="ps")
            for ch in range(2):
                t_sb = tailp.tile([128, nw_sg], f32, tag=f"tmp{ch}")
                nc.vector.tensor_tensor(t_sb[:], out2_sb[ch][:], rstd_ps[:],
                                        OP.mult)
                ln_sb = tailp.tile([128, nw_sg], bf16, tag=f"ln{ch}")
                nc.vector.tensor_tensor(ln_sb[:], t_sb[:], nmu_ps[:], OP.add)
                nc.tensor.matmul(h1_ps[:],
                                 cs["W1T"][:, ch * 128:(ch + 1) * 128],
                                 ln_sb[:], start=(ch == 0), stop=(ch == 1))
            h1_sb = tailp.tile([128, nw_sg], bf16, tag="h1")
            nc.scalar.activation(h1_sb[:], h1_ps[:], AF.Gelu,
                                 bias=cs["b1v"][:, 0:1])
            lg_ps = psp.tile([1, nw_sg], f32, tag="ps")
            nc.tensor.matmul(lg_ps[:], cs["w2"][:], h1_sb[:],
                             start=True, stop=True)
            lg_sb = tailp.tile([1, nw_sg], f32, tag="lg")
            nc.vector.scalar_tensor_tensor(lg_sb[:], lg_ps[:], b2f,
                                           vld[0:1, w0:w0 + nw_sg],
                                           OP.add, OP.mult)
            nc.sync.dma_start(out[0:1, w0:w0 + nw_sg], lg_sb[:])


def kernel(x, mask, query, in_proj_w, in_proj_b, out_w, out_b,
           ln_g, ln_b, w1, b1, w2, b2, _trace=False):
    import concourse.tile as tile
    from concourse import bacc, mybir, bass_utils

    x = np.ascontiguousarray(np.asarray(x, np.float32))
    mask = np.asarray(mask)
    consts, b2f = _build_consts(
        np.asarray(query, np.float32), np.asarray(in_proj_w, np.float32),
        np.asarray(in_proj_b, np.float32), np.asarray(out_w, np.float32),
        np.asarray(out_b, np.float32), np.asarray(ln_g, np.float32),
        np.asarray(ln_b, np.float32), np.asarray(w1, np.float32),
        np.asarray(b1, np.float32), np.asarray(w2, np.float32),
        np.asarray(b2, np.float32))

    in_maps = []
    for c in range(NCORES):
        xs = np.ascontiguousarray(
            x.reshape(B, N * T, E)[c * BPC:(c + 1) * BPC].reshape(LT, E))
        ms = np.asarray(mask).reshape(B, N * T)[c * BPC:(c + 1) * BPC] \
            .reshape(LT)
        mpad = np.zeros(NMCOL * TT, np.float32)
        mpad[:LT] = (ms != 0).astype(np.float32)
        mcols = np.zeros((128, NMCOL * 4), np.float32)
        mcols[:TT, :] = np.repeat(mpad.reshape(NMCOL, TT).T, 4, axis=1)
        valid = (ms.reshape(LW, T) != 0).any(axis=1).astype(np.float32)
        im = {"x": xs, "mcols": mcols, "valid": valid.reshape(1, LW)}
        im.update(consts)
        in_maps.append(im)

    nc = bacc.Bacc("TRN2", target_bir_lowering=False, debug=False,
                   enable_asserts=False, num_devices=NCORES)
    f32 = mybir.dt.float32
    bf16 = mybir.dt.bfloat16
    d = {"x": nc.dram_tensor("x", [LT, E], f32, kind="ExternalInput").ap(),
         "mcols": nc.dram_tensor("mcols", [128, NMCOL * 4], f32,
                                 kind="ExternalInput").ap(),
         "valid": nc.dram_tensor("valid", [1, LW], f32,
                                 kind="ExternalInput").ap()}
    for name, shape, dt in CONST_SPECS:
        d[name] = nc.dram_tensor(name, shape, bf16 if dt == "bf16" else f32,
                                 kind="ExternalInput").ap()
    out = nc.dram_tensor("out", [1, LW], f32, kind="ExternalOutput").ap()

    with tile.TileContext(nc) as tc:
        _emit(tc, d, out, b2f)
    nc.compile()

    res = bass_utils.run_bass_kernel_spmd(
        nc, in_maps, core_ids=list(range(NCORES)), trace=_trace)
    if _trace:
        print(f"HW exec time: {res.exec_time_ns} ns")
        print("trace:", res.instructions_and_trace[1]
              if res.instructions_and_trace else None)
    outs = [res.results[c]["out"].reshape(BPC, N) for c in range(NCORES)]
    return np.concatenate(outs, axis=0).astype(np.float32)
